# revision 12
# baseline (speedup 1.0000x reference)
"""Trainium2 Bass kernel for the MDA head (mixture-density logpdf + logsumexp).

Math: for component m (CK=2000 total), with lower-triangular Cholesky L_m,
  maha(b,m) = ||L_m^{-1}(z_b - mu_m)||^2 = z P z - 2 h^T z + c,
  P_m = L_m^{-T} L_m^{-1},  h_m = P_m mu_m,  c_m = mu_m^T P_m mu_m.
So  logpdf + logpi + prior = G @ W^T  with
  G_b = [packed(z_i z_j), z, 1]                     (B, CTR)
  W_m = [packed_scaled(P_m), h_m, const_m]          (CK, CTR)
where packed runs over lower-triangular (i>=j) indices, off-diagonal P entries
carry a factor 2 (folded with the global -0.5 into the W coefficients), and
  const_m = -0.5*(c_m + logdet_m + D log 2pi) + logpi_m + prior_class(m).
The per-class logsumexp over K=2 runs on-device; classes never cross cores.

Sharding: 2000 components -> 8 cores x 250 (= 125 whole classes per core).
Each core computes S = G @ W_slice^T as a single PE-array matmul chain
(contract dim 8448 = 66 x 128 tiles) and the K=2 logsumexp epilogue.
"""

import os
import sys

import numpy as np

if "/opt/trn_rl_repo" not in sys.path:
    sys.path.insert(0, "/opt/trn_rl_repo")

B, C, K, D = 256, 1000, 2, 128
CK = C * K
NCORES = 8
CPC = C // NCORES          # classes per core = 125
MPC = CPC * K              # components per core = 250
TRI = D * (D + 1) // 2     # 8256 packed quadratic terms
CTR = TRI + D + 1          # 8385 contraction length
KTILES = (CTR + 127) // 128  # 66
CTRP = KTILES * 128        # 8448 padded
NCOLS = 256                # 250 components + 6 zero pad (>=256 for f32r full rate)
LOG2PI = float(np.log(2.0 * np.pi))

_TRI_I, _TRI_J = np.tril_indices(D)

# matmul operand dtype: "float32r" (full-rate fp32 PE mode) or "float32" (4x slower)
MM_DTYPE = os.environ.get("MDA_MM_DTYPE", "float32r")

_PROGRAM = None


def _build_program():
    import concourse.bacc as bacc
    import concourse.mybir as mybir
    import concourse.tile as tile

    f32 = mybir.dt.float32
    mm_dt = getattr(mybir.dt, MM_DTYPE)

    nc = bacc.Bacc("TRN2", target_bir_lowering=False)
    KW = B + NCOLS                 # 512 columns per k-tile: [g (256 b) | w (256)]
    gw = nc.dram_tensor("gw", [128, KTILES * KW], mm_dt, kind="ExternalInput")
    out = nc.dram_tensor("out", [B, CPC], f32, kind="ExternalOutput")

    CHUNK = int(os.environ.get("MDA_CHUNK", "6"))  # k-tiles per DMA chunk
    assert KTILES % CHUNK == 0
    NCHUNKS = KTILES // CHUNK

    with tile.TileContext(nc) as tc:
        with (
            tc.tile_pool(name="gp", bufs=1) as gpool,
            tc.tile_pool(name="pp", bufs=1, space="PSUM") as ppool,
            tc.tile_pool(name="ep", bufs=1) as epool,
        ):
            psum = [
                ppool.tile([128, NCOLS], f32, tag=f"ps{bt}", name=f"ps{bt}")
                for bt in range(2)
            ]
            for ch in range(NCHUNKS):
                # every chunk gets its own SBUF slot (whole gw is resident;
                # no slot reuse -> chunk DMAs carry no waits, matmuls one)
                gwtile = gpool.tile(
                    [128, CHUNK * KW], mm_dt, tag=f"gw{ch}", name=f"gwt{ch}"
                )
                nc.sync.dma_start(
                    gwtile[:], gw[:, ch * CHUNK * KW:(ch + 1) * CHUNK * KW]
                )
                for kk in range(CHUNK):
                    k = ch * CHUNK + kk
                    rhs = gwtile[:, kk * KW + B: kk * KW + B + NCOLS]
                    for bt in range(2):
                        lhsT = gwtile[
                            :, kk * KW + bt * 128: kk * KW + bt * 128 + 128
                        ]
                        nc.tensor.matmul(
                            psum[bt][:],
                            lhsT,
                            rhs,
                            start=(k == 0),
                            stop=(k == KTILES - 1),
                        )
            # epilogue: per-class logsumexp over the K=2 components.
            # column layout: [k=0 of 125 classes | k=1 of 125 classes | pad]
            for bt in range(2):
                a = psum[bt][:, 0:CPC]
                b = psum[bt][:, CPC:2 * CPC]
                sb = epool.tile([128, CPC], f32, tag=f"sb{bt}")
                nc.scalar.copy(sb[:], b)
                mx = epool.tile([128, CPC], f32, tag=f"mx{bt}")
                nc.vector.tensor_max(mx[:], a, sb[:])
                t1 = epool.tile([128, CPC], f32, tag=f"t1{bt}")
                t2 = epool.tile([128, CPC], f32, tag=f"t2{bt}")
                nc.vector.tensor_sub(t1[:], a, mx[:])
                nc.vector.tensor_sub(t2[:], sb[:], mx[:])
                nc.scalar.activation(t1[:], t1[:], mybir.ActivationFunctionType.Exp)
                nc.scalar.activation(t2[:], t2[:], mybir.ActivationFunctionType.Exp)
                nc.vector.tensor_add(t1[:], t1[:], t2[:])
                nc.scalar.activation(t1[:], t1[:], mybir.ActivationFunctionType.Ln)
                nc.vector.tensor_add(t1[:], t1[:], mx[:])
                nc.sync.dma_start(out[bt * 128:(bt + 1) * 128, :], t1[:])
    nc.compile()
    return nc


def _get_program():
    global _PROGRAM
    if _PROGRAM is None:
        _PROGRAM = _build_program()
    return _PROGRAM


def _ktile_layout(x):
    """(CTRP, N) -> (128, KTILES, N): partition p holds row p of every k-tile."""
    n = x.shape[1]
    return x.reshape(KTILES, 128, n).transpose(1, 0, 2)


# stash of the last run's results object (exec_time_ns etc.) for test harnesses
LAST_RUN = None


def kernel(z, mu, logits_pi, covL, logits_prior):
    from concourse.bass_utils import run_bass_kernel_spmd

    # ---- host precompute (fp64): precision matrices and affine folding ----
    L = covL.reshape(CK, D, D).astype(np.float64)
    eye = np.eye(D, dtype=np.float64)
    Linv = np.linalg.solve(L, np.broadcast_to(eye, (CK, D, D)))
    P = np.matmul(Linv.transpose(0, 2, 1), Linv)          # (CK, D, D)
    mu_f = mu.reshape(CK, D).astype(np.float64)
    h = np.einsum("mij,mj->mi", P, mu_f)                   # (CK, D)
    c = np.einsum("mi,mi->m", mu_f, h)                     # (CK,)
    logdet = 2.0 * np.sum(np.log(np.diagonal(L, axis1=1, axis2=2)), axis=1)
    lp = logits_pi.astype(np.float64)                      # (C, K)
    lse = np.max(lp, axis=1, keepdims=True)
    lse = lse + np.log(np.sum(np.exp(lp - lse), axis=1, keepdims=True))
    logpi = (lp - lse).reshape(CK)
    prior = np.repeat(logits_prior.astype(np.float64), K)  # (CK,)
    const = -0.5 * (c + logdet + D * LOG2PI) + logpi + prior

    scale = np.where(_TRI_I == _TRI_J, -0.5, -1.0)         # fold -0.5 and symmetry
    Wq = P[:, _TRI_I, _TRI_J] * scale                      # (CK, TRI)
    Wfull = np.concatenate([Wq, h, const[:, None]], axis=1).astype(np.float32)

    zf = z.astype(np.float64)
    zz = zf[:, _TRI_I] * zf[:, _TRI_J]                     # (B, TRI)
    Gfull = np.concatenate(
        [zz, zf, np.ones((B, 1))], axis=1
    ).astype(np.float32)                                   # (B, CTR)

    Gt = np.zeros((CTRP, B), np.float32)
    Gt[:CTR] = Gfull.T
    GtK = _ktile_layout(Gt)                                # (128, KTILES, 256)

    in_maps = []
    for core in range(NCORES):
        cls = np.arange(CPC) + CPC * core
        comp_idx = np.concatenate([cls * K, cls * K + 1])  # k=0 block, k=1 block
        Wt = np.zeros((CTRP, NCOLS), np.float32)
        Wt[:CTR, :MPC] = Wfull[comp_idx].T
        gws = np.empty((128, KTILES, B + NCOLS), np.float32)
        gws[:, :, :B] = GtK
        gws[:, :, B:] = _ktile_layout(Wt)
        in_maps.append({"gw": gws.reshape(128, KTILES * (B + NCOLS))})

    nc = _get_program()
    res = run_bass_kernel_spmd(nc, in_maps, core_ids=list(range(NCORES)))
    global LAST_RUN
    LAST_RUN = res
    return np.concatenate(
        [res.results[i]["out"] for i in range(NCORES)], axis=1
    ).astype(np.float32)


# revision 17
# speedup vs baseline: 1.7326x; 1.7326x over previous
"""Trainium2 Bass kernel for the MDA head (mixture-density logpdf + logsumexp).

Math: for component m (CK=2000 total), with lower-triangular Cholesky L_m,
  maha(b,m) = ||L_m^{-1}(z_b - mu_m)||^2 = z P z - 2 h^T z + c,
  P_m = L_m^{-T} L_m^{-1},  h_m = P_m mu_m,  c_m = mu_m^T P_m mu_m.
So  logpdf + logpi + prior = G @ W^T  with
  G_b = [packed(z_i z_j), z, 1]                     (B, CTR)
  W_m = [packed_scaled(P_m), h_m, const_m]          (CK, CTR)
where packed runs over lower-triangular (i>=j) indices, off-diagonal P entries
carry a factor 2 (folded with the global -0.5 into the W coefficients), and
  const_m = -0.5*(c_m + logdet_m + D log 2pi) + logpi_m + prior_class(m).
The per-class logsumexp over K=2 runs on-device; classes never cross cores.

Sharding: 2000 components -> 8 cores x 250 (= 125 whole classes per core).
Each core computes S = G @ W_slice^T as a single PE-array matmul chain
(contract dim 8448 = 66 x 128 tiles) and the K=2 logsumexp epilogue.
"""

import os
import sys

import numpy as np

if "/opt/trn_rl_repo" not in sys.path:
    sys.path.insert(0, "/opt/trn_rl_repo")

B, C, K, D = 256, 1000, 2, 128
CK = C * K
NCORES = 8
CPC = C // NCORES          # classes per core = 125
MPC = CPC * K              # components per core = 250
TRI = D * (D + 1) // 2     # 8256 packed quadratic terms
CTR = TRI + D + 1          # 8385 contraction length
KTILES = (CTR + 127) // 128  # 66
CTRP = KTILES * 128        # 8448 padded
NCOLS = 256                # 250 components + 6 zero pad (>=256 for f32r full rate)
LOG2PI = float(np.log(2.0 * np.pi))

_TRI_I, _TRI_J = np.tril_indices(D)

# matmul operand dtype: "bfloat16" (fast; accuracy preserved by identity-split),
# "float32r" (full-rate fp32 PE mode) or "float32" (4x slower)
MM_DTYPE = os.environ.get("MDA_MM_DTYPE", "bfloat16")

_PROGRAM = None


def _build_program():
    import concourse.bacc as bacc
    import concourse.mybir as mybir
    import concourse.tile as tile

    f32 = mybir.dt.float32
    mm_dt = getattr(mybir.dt, MM_DTYPE)

    nc = bacc.Bacc("TRN2", target_bir_lowering=False)
    KW = B + NCOLS                 # 512 columns per k-tile: [g (256 b) | w (256)]
    gw = nc.dram_tensor("gw", [128, KTILES * KW], mm_dt, kind="ExternalInput")
    s0 = nc.dram_tensor("s0", [B, 1], f32, kind="ExternalInput")
    out = nc.dram_tensor("out", [B, CPC], f32, kind="ExternalOutput")

    CHUNK = int(os.environ.get("MDA_CHUNK", "6"))  # k-tiles per DMA chunk
    assert KTILES % CHUNK == 0
    NCHUNKS = KTILES // CHUNK

    with tile.TileContext(nc) as tc:
        with (
            tc.tile_pool(name="gp", bufs=1) as gpool,
            tc.tile_pool(name="pp", bufs=1, space="PSUM") as ppool,
            tc.tile_pool(name="ep", bufs=1) as epool,
        ):
            psum = [
                ppool.tile([128, NCOLS], f32, tag=f"ps{bt}", name=f"ps{bt}")
                for bt in range(2)
            ]
            for ch in range(NCHUNKS):
                # every chunk gets its own SBUF slot (whole gw is resident;
                # no slot reuse -> chunk DMAs carry no waits, matmuls one)
                gwtile = gpool.tile(
                    [128, CHUNK * KW], mm_dt, tag=f"gw{ch}", name=f"gwt{ch}"
                )
                nc.sync.dma_start(
                    gwtile[:], gw[:, ch * CHUNK * KW:(ch + 1) * CHUNK * KW]
                )
                for kk in range(CHUNK):
                    k = ch * CHUNK + kk
                    rhs = gwtile[:, kk * KW + B: kk * KW + B + NCOLS]
                    for bt in range(2):
                        lhsT = gwtile[
                            :, kk * KW + bt * 128: kk * KW + bt * 128 + 128
                        ]
                        nc.tensor.matmul(
                            psum[bt][:],
                            lhsT,
                            rhs,
                            start=(k == 0),
                            stop=(k == KTILES - 1),
                        )
            # epilogue: per-class logsumexp over the K=2 components, plus the
            # per-sample bias s0 = -0.5||z||^2 - 0.5 D log2pi (identity part
            # of the precision matrices, kept in exact fp32).
            # column layout: [k=0 of 125 classes | k=1 of 125 classes | pad]
            s0t = epool.tile([128, 2], f32, tag="s0", name="s0t")
            nc.sync.dma_start(s0t[:], s0[:].rearrange("(t p) o -> p (t o)", p=128))
            for bt in range(2):
                a = psum[bt][:, 0:CPC]
                b = psum[bt][:, CPC:2 * CPC]
                sb = epool.tile([128, CPC], f32, tag=f"sb{bt}")
                nc.scalar.copy(sb[:], b)
                mx = epool.tile([128, CPC], f32, tag=f"mx{bt}")
                nc.vector.tensor_max(mx[:], a, sb[:])
                t1 = epool.tile([128, CPC], f32, tag=f"t1{bt}")
                t2 = epool.tile([128, CPC], f32, tag=f"t2{bt}")
                nc.vector.tensor_sub(t1[:], a, mx[:])
                nc.vector.tensor_sub(t2[:], sb[:], mx[:])
                nc.scalar.activation(t1[:], t1[:], mybir.ActivationFunctionType.Exp)
                nc.scalar.activation(t2[:], t2[:], mybir.ActivationFunctionType.Exp)
                nc.vector.tensor_add(t1[:], t1[:], t2[:])
                nc.scalar.activation(t1[:], t1[:], mybir.ActivationFunctionType.Ln)
                nc.vector.scalar_tensor_tensor(
                    t1[:], t1[:], s0t[:, bt:bt + 1], mx[:],
                    op0=mybir.AluOpType.add, op1=mybir.AluOpType.add,
                )
                nc.sync.dma_start(out[bt * 128:(bt + 1) * 128, :], t1[:])
    nc.compile()
    return nc


def _get_program():
    global _PROGRAM
    if _PROGRAM is None:
        _PROGRAM = _build_program()
    return _PROGRAM


def _ktile_layout(x):
    """(CTRP, N) -> (128, KTILES, N): partition p holds row p of every k-tile."""
    n = x.shape[1]
    return x.reshape(KTILES, 128, n).transpose(1, 0, 2)


# stash of the last run's results object (exec_time_ns etc.) for test harnesses
LAST_RUN = None


def kernel(z, mu, logits_pi, covL, logits_prior):
    from concourse.bass_utils import run_bass_kernel_spmd

    # ---- host precompute (fp64): precision matrices and affine folding ----
    L = covL.reshape(CK, D, D).astype(np.float64)
    eye = np.eye(D, dtype=np.float64)
    Linv = np.linalg.solve(L, np.broadcast_to(eye, (CK, D, D)))
    P = np.matmul(Linv.transpose(0, 2, 1), Linv)          # (CK, D, D)
    mu_f = mu.reshape(CK, D).astype(np.float64)
    h = np.einsum("mij,mj->mi", P, mu_f)                   # (CK, D)
    c = np.einsum("mi,mi->m", mu_f, h)                     # (CK,)
    logdet = 2.0 * np.sum(np.log(np.diagonal(L, axis1=1, axis2=2)), axis=1)
    lp = logits_pi.astype(np.float64)                      # (C, K)
    lse = np.max(lp, axis=1, keepdims=True)
    lse = lse + np.log(np.sum(np.exp(lp - lse), axis=1, keepdims=True))
    logpi = (lp - lse).reshape(CK)
    prior = np.repeat(logits_prior.astype(np.float64), K)  # (CK,)
    # identity-split: P = I + E. The -0.5||z||^2 - 0.5 D log2pi part is added
    # in fp32 via the epilogue bias s0; only the small residual E (and the
    # small per-component constant) rides the (possibly bf16) matmul.
    const = -0.5 * (c + logdet) + logpi + prior

    E = P - np.eye(D)[None]
    scale = np.where(_TRI_I == _TRI_J, -0.5, -1.0)         # fold -0.5 and symmetry
    Wq = E[:, _TRI_I, _TRI_J] * scale                      # (CK, TRI)
    Wfull = np.concatenate([Wq, h, const[:, None]], axis=1)

    zf = z.astype(np.float64)
    zz = zf[:, _TRI_I] * zf[:, _TRI_J]                     # (B, TRI)
    Gfull = np.concatenate([zz, zf, np.ones((B, 1))], axis=1)  # (B, CTR)
    s0 = (-0.5 * (zf * zf).sum(axis=1) - 0.5 * D * LOG2PI).astype(np.float32)

    import ml_dtypes

    np_mm = {"bfloat16": ml_dtypes.bfloat16}.get(MM_DTYPE, np.float32)
    Gt = np.zeros((CTRP, B), np_mm)
    Gt[:CTR] = Gfull.T.astype(np_mm)
    GtK = _ktile_layout(Gt)                                # (128, KTILES, 256)

    in_maps = []
    for core in range(NCORES):
        cls = np.arange(CPC) + CPC * core
        comp_idx = np.concatenate([cls * K, cls * K + 1])  # k=0 block, k=1 block
        Wt = np.zeros((CTRP, NCOLS), np_mm)
        Wt[:CTR, :MPC] = Wfull[comp_idx].T.astype(np_mm)
        gws = np.empty((128, KTILES, B + NCOLS), np_mm)
        gws[:, :, :B] = GtK
        gws[:, :, B:] = _ktile_layout(Wt)
        in_maps.append({
            "gw": gws.reshape(128, KTILES * (B + NCOLS)),
            "s0": s0[:, None],
        })

    nc = _get_program()
    res = run_bass_kernel_spmd(nc, in_maps, core_ids=list(range(NCORES)))
    global LAST_RUN
    LAST_RUN = res
    return np.concatenate(
        [res.results[i]["out"] for i in range(NCORES)], axis=1
    ).astype(np.float32)
